# revision 30
# baseline (speedup 1.0000x reference)
"""Trainium2 Bass kernel for nn_CubeSimulator.

Reference computation: a 128^3 spatial grid is rotated (Rz(sky_rot) then
Rx(inclination)), a rotation-curve velocity field and an exponential-disk
intensity field are evaluated, an 80-channel Gaussian KDE over the
line-of-sight velocity reduces the third grid axis, and the [80,128,128]
cube is avg-pooled (5,4,4) to [16,32,32].

Kernel strategy (v3)
--------------------
* erf-collapsed KDE: a 5-channel pooled group sum of Gaussians at spacing
  dz << sigma is a midpoint-rule sum, equal to an erf difference with
  aliasing error ~e^{-pi^2 sig^2/dz^2} once the erf width is corrected to
  sig_e = sqrt(sig^2 - dz^2/6):
      sum_{r=0..4} exp(-(z_{5m+r}-vz)^2/sig^2)
        ~= C * [erf((e_{m+1}-vz)/sig_e) - erf((e_m-vz)/sig_e)]
  (~3e-5 max error).  80 exp channels collapse to <=15 erf evaluations.
* edge-slot sparsity: bf16 erf saturates to +-1 for |arg|>2.2, so per
  column tile only edges within the tile's vz range (+- margin) matter;
  live edges form a contiguous range [mlo..mhi].  The SPMD program has S =
  max-over-cores live-edge *slots* per tile; each core loads its own edge
  values into the sm bias columns and the host un-maps psum slots to
  physical velocity groups per core.  Cells are re-sharded globally by vz
  so each (core, tile) has a narrow vz span.
* +- PE accumulation: per edge slot only P_s = erf_s * src is formed; the
  group slot between slots s-1,s accumulates (+P_s) and (-P_{s-1}) via
  +1/-1 stationary vectors; boundary slots use (+src) for the saturated
  side.  PSUM start/stop handles the velocity pooling, no elementwise
  differences.
* k-window packing: src has a Gaussian vertical profile; per sky column a
  64-step |rot_z| window holds all non-negligible intensity.  Two sky
  points of one output pool cell pack into one 128-partition column.
* Point symmetry: (i,j,k) -> (-i,-j,-k) negates vz and preserves src, so
  only rows i<64 are computed; the host mirrors the pooled output.
"""

import sys

for _p in ("/opt/trn_rl_repo",):
    if _p not in sys.path:
        sys.path.insert(0, _p)

import numpy as np
import ml_dtypes

# ---------------- problem constants (compile-time, model-intrinsic) --------
IMAGE_RES = 128
VEL_RES = 80
VEL_UP = 5
IMG_UP = 4
N_CORES = 8
HALF_I = IMAGE_RES // 2            # 64 computed rows
KWIN = 64                          # k-window length (2 points/column)
COLS = 512                         # packed columns per core (1024 points)
TCOLS = COLS // 2                  # columns per sparsity tile
CELLS_PER_TILE = 32
CUBE_FOV = 1000.0
M_TO_PC = 1.0 / 3.086e16
V_MAX_PC = np.float32(200000.0 * M_TO_PC)
R_C = np.float32(0.1 * CUBE_FOV)
R_D = np.float32(0.3 * CUBE_FOV)
H_Z = np.float32(0.05 * CUBE_FOV)
VEL_MIN = -300000.0
VEL_MAX = 300000.0
N_GROUPS = VEL_RES // VEL_UP       # 16
N_EDGES = N_GROUPS + 1             # 17
MARGIN = 2.5                       # erf-saturation margin (sig_e units)

_INV_RD2 = 1.0 / (float(R_D) * float(R_D))
_NEG_H = float(-1.0 / (2.0 * float(H_Z) * float(H_Z)))
_RC2 = float(R_C) * float(R_C)
_EPS_R2D2 = np.float32(1e-25)

# sm scalar-column layout: [ nsz | ciz | rc2 | negh | ird2 | edge slots... ]
_C_NSZ = 0
_C_CIZ = 1
_C_RC2 = 2
_C_NEGH = 3
_C_IRD2 = 4
_C_EDG = 5                 # then S0 cols for tile0 slots, S1 for tile1

_CACHE = {}


def _build_program(S0, S1):
    from concourse import bacc, mybir, tile

    f32 = mybir.dt.float32
    bf16 = mybir.dt.bfloat16
    AF = mybir.ActivationFunctionType
    OP = mybir.AluOpType

    SM_COLS = _C_EDG + S0 + S1
    SLOTS = (S0, S1)
    # globally unique psum regions: tile0 slots 0..S0, tile1 S0+1..S0+S1+1
    n_regions = (S0 + 1) + (S1 + 1)
    assert n_regions <= 24, "edge-slot count exceeds PSUM capacity"
    nbanks = (n_regions + 2) // 3
    nrows_out = 3 * nbanks
    rbase = (0, S0 + 1)

    nc = bacc.Bacc(None)

    pk_d = nc.dram_tensor("pk", [128, 4 * COLS], f32, kind="ExternalInput")
    sm_d = nc.dram_tensor("sm", [128, SM_COLS], f32, kind="ExternalInput")
    ones_d = nc.dram_tensor("ones", [128, 64], bf16, kind="ExternalInput")
    out_d = nc.dram_tensor("out", [nrows_out, COLS], f32,
                           kind="ExternalOutput")

    with tile.TileContext(nc) as tc:
        with (
            tc.tile_pool(name="inp", bufs=1) as inp,
            tc.tile_pool(name="fld", bufs=1) as fld,
            tc.tile_pool(name="ep", bufs=4) as ep,
            tc.tile_pool(name="pp", bufs=6) as pp,
            tc.tile_pool(name="psum", bufs=1, space="PSUM") as psum,
            tc.tile_pool(name="ob", bufs=4) as obp,
        ):
            pk = inp.tile([128, 4 * COLS], f32)
            sm = inp.tile([128, SM_COLS], f32)
            ones = inp.tile([128, 64], bf16)
            # pk quarters spread over three queues, ordered by need;
            # pa/pb are split into chunk-sized halves so the chunk-0 field
            # chain starts as soon as its half lands
            HALF = COLS // 2
            for _c, _h, _q in ((0, 0, nc.sync), (1, 0, nc.sync),
                               (2, 0, nc.gpsimd), (2, 1, nc.gpsimd),
                               (0, 1, nc.sync), (1, 1, nc.sync),
                               (3, 0, nc.scalar), (3, 1, nc.scalar)):
                lo = _c * COLS + _h * HALF
                _q.dma_start(pk[:, lo:lo + HALF], pk_d[:, lo:lo + HALF])
            nc.gpsimd.dma_start(sm[:], sm_d[:])
            nc.gpsimd.dma_start(ones[:], ones_d[:])

            pa = pk[:, 0 * COLS:1 * COLS]
            pb = pk[:, 1 * COLS:2 * COLS]
            prx2 = pk[:, 2 * COLS:3 * COLS]
            pc = pk[:, 3 * COLS:4 * COLS]

            def col(i):
                return sm[:, i:i + 1]

            V = nc.vector
            G = nc.gpsimd
            S = nc.scalar

            roty = fld.tile([128, COLS], f32)
            rotz = fld.tile([128, COLS], f32)
            y2 = fld.tile([128, COLS], f32)
            r2d2 = fld.tile([128, COLS], f32)
            z2 = fld.tile([128, COLS], f32)
            q = fld.tile([128, COLS], f32)
            qc = fld.tile([128, COLS], f32)
            den = fld.tile([128, COLS], f32)
            rec = fld.tile([128, COLS], f32)
            u2 = fld.tile([128, 2 * COLS], f32)    # [ u | r2d2*invRD2 ]
            sus2 = fld.tile([128, 2 * COLS], f32)  # [ su | slq ]
            sarg = fld.tile([128, COLS], f32)
            sg = fld.tile([128, COLS], f32)
            tr = fld.tile([128, COLS], f32)
            vzt = fld.tile([128, COLS], f32)
            src = fld.tile([128, COLS], bf16)
            dummy = fld.tile([128, COLS], bf16)

            G.memset(dummy[:], 0.0)
            # pre-trigger the Sqrt act-table load while DMAs are in flight
            S.activation(sarg[:, 0:1], sm[:, _C_RC2:_C_RC2 + 1], AF.Sqrt)

            CHUNKS = ((0, G), (1, V))

            def sl_of(c):
                return slice(c * HALF, (c + 1) * HALF)

            for c, E in CHUNKS:
                s = sl_of(c)
                E.tensor_scalar_add(roty[:, s], pa[:, s], col(_C_NSZ))
                E.tensor_scalar_add(rotz[:, s], pb[:, s], col(_C_CIZ))
                E.tensor_mul(y2[:, s], roty[:, s], roty[:, s])
                E.tensor_add(r2d2[:, s], y2[:, s], prx2[:, s])
                E.tensor_mul(z2[:, s], rotz[:, s], rotz[:, s])
                E.tensor_add(q[:, s], r2d2[:, s], z2[:, s])
                if E is V:
                    E.scalar_tensor_tensor(den[:, s], q[:, s], col(_C_RC2),
                                           r2d2[:, s], op0=OP.add,
                                           op1=OP.mult)
                else:
                    E.tensor_scalar_add(qc[:, s], q[:, s], col(_C_RC2))
                    E.tensor_mul(den[:, s], qc[:, s], r2d2[:, s])

            # PE warmup + zero-init every used psum region (skipped group
            # slots keep these zeros; real slots reset via start=True)
            psb = [psum.tile([128, COLS], f32, name=f"acc{b}")
                   for b in range(nbanks)]
            for b in range(nbanks):
                for r in range(3):
                    nc.tensor.matmul(psb[b][32 * r:32 * r + 32, :],
                                     ones[:, 0:32], dummy[:, :],
                                     start=True, stop=True,
                                     skip_group_check=True)

            # u2 layout per chunk c: [ u_c (256) | r2d2_c/R_D^2 (256) ] so
            # each half can take one contiguous sqrt as soon as it's ready.
            def u_sl(c):
                return slice(c * COLS, c * COLS + HALF)

            def r_sl(c):
                return slice(c * COLS + HALF, (c + 1) * COLS)

            for c, _E in CHUNKS:
                # idle-window ACT; Copy is in every act table
                S.activation(u2[:, r_sl(c)], r2d2[:, sl_of(c)], AF.Copy,
                             scale=_INV_RD2)
            for c, _E in CHUNKS:
                s = sl_of(c)
                V.reciprocal_approx_fast(rec[:, s], den[:, s])
            for c, E in CHUNKS:
                s = sl_of(c)
                E.tensor_mul(u2[:, u_sl(c)], q[:, s], rec[:, s])
                # [su_c | slq_c] = sqrt([u_c | r2d2_c/R_D^2]), Sqrt preloaded
                S.activation(sus2[:, c * COLS:(c + 1) * COLS],
                             u2[:, c * COLS:(c + 1) * COLS], AF.Sqrt)

            def su_part(c):
                return sus2[:, c * COLS:c * COLS + HALF]

            def slq_part(c):
                return sus2[:, c * COLS + HALF:(c + 1) * COLS]

            for c, E in CHUNKS:
                s = sl_of(c)
                # sarg first so the src sigmoid becomes ready early
                if E is V:
                    E.scalar_tensor_tensor(sarg[:, s], z2[:, s], col(_C_NEGH),
                                           slq_part(c), op0=OP.mult,
                                           op1=OP.subtract)
                else:
                    E.tensor_scalar_mul(qc[:, s], z2[:, s], col(_C_NEGH))
                    E.tensor_sub(sarg[:, s], qc[:, s], slq_part(c))

            # src = exp(sarg) via sigmoid (erf's own act table -> only one
            # mid-stream table switch): e^-s = 1/sigmoid(s) - 1
            # per tile so tile0's erf products unblock immediately
            for t in (0, 1):
                ts_ = slice(t * TCOLS, (t + 1) * TCOLS)
                S.activation(sg[:, ts_], sarg[:, ts_], AF.Sigmoid, scale=-1.0)
                V.reciprocal_approx_fast(tr[:, ts_], sg[:, ts_])
                G.tensor_scalar_add(src[:, ts_], tr[:, ts_], -1.0)

            for c, E in CHUNKS:
                s = sl_of(c)
                E.tensor_mul(vzt[:, s], su_part(c), pc[:, s])

            # ---- KDE: erf edge slots + (+-1)-stationary PSUM streams ----
            onesp = ones[:, 0:32]
            onesn = ones[:, 32:64]

            def tslice(t):
                return slice(t * TCOLS, (t + 1) * TCOLS)

            # group slot (t, g) -> psum bank g//3, rows 32*(g%3), tile cols
            started = set()

            def mm(t, g, stat, mov, stop):
                R = rbase[t] + g
                b, r = R // 3, R % 3
                st = (t, g) not in started
                started.add((t, g))
                nc.tensor.matmul(
                    psb[b][32 * r:32 * r + 32, tslice(t)], stat, mov,
                    start=st, stop=stop, skip_group_check=True)

            # bank completion bookkeeping
            bank_need = [0] * nbanks
            for t in (0, 1):
                for g in range(SLOTS[t] + 1):
                    bank_need[(rbase[t] + g) // 3] += 1
            bank_got = [0] * nbanks
            banks_done = [0]
            deferred = []

            def emit_bank_out(b, on_act, q):
                # each bank's regions belong to a single tile -> only move
                # that tile's column half (the other half is warmup zeros)
                tset = {0 if r <= S0 else 1 for r in range(3 * b, 3 * b + 3)
                        if r < n_regions}
                cs = tslice(tset.pop()) if len(tset) == 1 else slice(0, COLS)
                ot = obp.tile([128, COLS], f32, tag="ob", name=f"ot{b}")
                if on_act:
                    # ACT is idle after the last erf
                    S.activation(ot[0:96, cs], psb[b][0:96, cs], AF.Copy)
                else:
                    V.tensor_copy(ot[0:96, cs], psb[b][0:96, cs])
                q.dma_start(out_d[3 * b:3 * b + 3, cs], ot[0:96:32, cs])

            def note_stop(t, g):
                b = (rbase[t] + g) // 3
                bank_got[b] += 1
                if bank_got[b] == bank_need[b]:
                    banks_done[0] += 1
                    # defer the last two banks so their copies don't steal
                    # DVE priority from the final erf products
                    if banks_done[0] <= nbanks - 2:
                        emit_bank_out(b, False,
                                      (nc.sync, nc.gpsimd)[banks_done[0] % 2])
                    else:
                        deferred.append(b)

            # group slot 0 and the final slot S_t of each tile get their
            # (+src) streams as soon as src exists; the final slot is then
            # closed by its (-P) stream right after the last erf product
            for t in (0, 1):
                mm(t, 0, onesp, src[:, tslice(t)], False)
                mm(t, SLOTS[t], onesp, src[:, tslice(t)], False)

            nerf = 0
            for t in (0, 1):
                ts = tslice(t)
                for s_idx in range(SLOTS[t]):
                    E_t = ep.tile([128, TCOLS], bf16, tag=f"E{t}")
                    S.activation(E_t[:], vzt[:, ts], AF.Erf,
                                 bias=col(_C_EDG + (0 if t == 0 else S0)
                                          + s_idx),
                                 scale=-1.0)
                    P_t = pp.tile([128, TCOLS], bf16, tag=f"P{t}")
                    n_left = (S0 + S1) - nerf
                    eng = V if n_left == 1 else (V if nerf % 2 else G)
                    eng.tensor_mul(P_t[:], E_t[:], src[:, ts])
                    nerf += 1
                    # +P_s closes group slot s; -P_s joins group slot s+1
                    # (already opened by +src for the final slot)
                    mm(t, s_idx, onesp, P_t[:, :], True)
                    note_stop(t, s_idx)
                    mm(t, s_idx + 1, onesn, P_t[:, :],
                       s_idx + 1 == SLOTS[t])
                    if s_idx + 1 == SLOTS[t]:
                        note_stop(t, SLOTS[t])
            for i, b in enumerate(deferred):
                # DVE copies (it is free after the last erf product);
                # alternate DMA queues, last on the faster sync queue
                emit_bank_out(b, False, (nc.gpsimd, nc.sync)[i % 2])

    nc.finalize()
    return nc


def _plan(inclination, sky_rot, line_broadening):
    """Host-side per-input planning: cell vz ranges, global re-sharding,
    per-(core,tile) edge windows, packed input tensors."""
    f32 = np.float32
    inc = f32(inclination)
    rot = f32(sky_rot)
    lb = f32(line_broadening)
    ci, si = f32(np.cos(inc)), f32(np.sin(inc))
    cr, sr = f32(np.cos(rot)), f32(np.sin(rot))

    lin = np.linspace(-CUBE_FOV, CUBE_FOV, IMAGE_RES, dtype=f32)
    dgrid = f32(lin[1] - lin[0])
    zl = np.linspace(f32(VEL_MIN * M_TO_PC), f32(VEL_MAX * M_TO_PC),
                     VEL_RES, dtype=f32)
    dz = float(zl[-1] - zl[0]) / (VEL_RES - 1)
    sig = float(lb)
    sig_e = f32(np.sqrt(sig * sig - dz * dz / 6.0))

    # dropped vertical src mass outside the 64-step window; measured output
    # error stays ~20x below it (drops hit dim pixels), so 2e-2 here keeps
    # rel err under ~1.5e-2 even for near-edge-on inclinations.
    t_keep = (KWIN / 2) * abs(float(ci)) * float(dgrid)
    eps_drop = np.exp(-t_keep * t_keep / (2.0 * float(H_Z) ** 2))
    if eps_drop > 2e-2:
        raise RuntimeError(
            f"k-window packing invalid for inclination={inc} "
            f"(eps_drop={eps_drop:.2e}); phi=1 fallback not built")

    edges = np.empty(N_EDGES, dtype=np.float64)
    edges[0] = zl[0] - dz / 2
    for m in range(1, N_GROUPS):
        edges[m] = (float(zl[5 * m - 1]) + float(zl[5 * m])) / 2
    edges[N_GROUPS] = zl[-1] + dz / 2
    edges = (edges - edges[::-1]) / 2
    ep_n = edges / float(sig_e)          # edge values in vzt units

    vmax_proj = abs(float(si)) * float(V_MAX_PC)
    margin = (float(edges[N_GROUPS]) - vmax_proj) / float(sig_e)
    if margin < 3.0:     # bf16 erf saturates to exactly 1.0 beyond ~2.2
        raise RuntimeError(
            f"outer erf edge not saturated (margin={margin:.2f} sigma)")

    # ---- per-cell geometry and vz ranges ----
    io = np.repeat(np.arange(16), 32)
    jo = np.tile(np.arange(32), 16)
    di = np.repeat(np.arange(4), 4)[None, :]
    dj = np.tile(np.arange(4), 4)[None, :]
    xi = lin[(io[:, None] * 4 + di)].astype(f32)     # [512, 16]
    yj = lin[(jo[:, None] * 4 + dj)].astype(f32)
    y1 = (sr * xi + cr * yj).astype(f32)
    rotx = (cr * xi - sr * yj).astype(f32)
    kc = (-si * y1 / ci - lin[0]) / dgrid
    k0 = np.clip(np.round(kc - KWIN / 2), 0,
                 IMAGE_RES - KWIN).astype(np.int64)
    pidx = np.arange(KWIN, dtype=f32)
    zk = (lin[0] + (k0[..., None] + pidx) * dgrid).astype(f32)  # [512,16,64]
    roty_h = (ci * y1)[..., None] - si * zk
    rotz_h = (si * y1)[..., None] + ci * zk
    r2d2_h = roty_h * roty_h + (rotx * rotx)[..., None] + 1e-25
    q_h = r2d2_h + rotz_h * rotz_h
    su_h = np.sqrt(q_h / ((q_h + float(R_C) ** 2) * r2d2_h))
    Cp = (-si * V_MAX_PC * rotx / sig_e).astype(f32)
    vzt_h = Cp[..., None] * su_h                     # [512, 16, 64]
    cell_min = vzt_h.min(axis=(1, 2))
    cell_max = vzt_h.max(axis=(1, 2))

    # ---- global sort, blocks of 32 cells, S per block ----
    order = np.argsort(cell_min + cell_max, kind="stable")
    blocks = [order[32 * b:32 * b + 32] for b in range(16)]

    def block_window(cells_idx):
        vmin = cell_min[cells_idx].min()
        vmax = cell_max[cells_idx].max()
        live = [m for m in range(1, N_GROUPS)
                if vmin - MARGIN <= ep_n[m] <= vmax + MARGIN]
        if live:
            return live[0], live[-1]
        # vz range inside one group: first edge above the range
        m = int(np.searchsorted(ep_n, vmax + MARGIN))
        m = min(max(m, 1), N_GROUPS - 1)
        return m, m

    wins = [block_window(b) for b in blocks]
    sizes = [w[1] - w[0] + 1 for w in wins]
    # big-S blocks to tile slot 0 (one per core), small to slot 1
    bo = sorted(range(16), key=lambda b: -sizes[b])
    slot_blocks = [bo[:8], bo[8:]]
    S0 = max(sizes[b] for b in slot_blocks[0])
    S1 = max(sizes[b] for b in slot_blocks[1])

    SM_COLS = _C_EDG + S0 + S1
    sm_base = np.zeros((128, SM_COLS), dtype=f32)
    pmod = (np.arange(128) % KWIN).astype(f32)
    sm_base[:, _C_NSZ] = (-si * dgrid) * pmod
    sm_base[:, _C_CIZ] = (ci * dgrid) * pmod
    sm_base[:, _C_RC2] = f32(_RC2)
    sm_base[:, _C_NEGH] = f32(_NEG_H)
    sm_base[:, _C_IRD2] = f32(_INV_RD2)

    ones = np.empty((128, 64), dtype=ml_dtypes.bfloat16)
    ones[:, 0:32] = 1.0
    ones[:, 32:64] = -1.0

    in_maps = []
    core_meta = []
    for core in range(N_CORES):
        sm = sm_base.copy()
        cells_t = []
        mlos = []
        for t, Sx in ((0, S0), (1, S1)):
            b = slot_blocks[t][core]
            mlo, mhi = wins[b]
            while mhi - mlo + 1 < Sx:
                if mhi < N_GROUPS - 1:
                    mhi += 1
                else:
                    mlo -= 1
            assert mlo >= 1 and mhi <= N_GROUPS - 1
            base = _C_EDG if t == 0 else _C_EDG + S0
            for s_i in range(Sx):
                sm[:, base + s_i] = f32(ep_n[mlo + s_i])
            cells_t.append(blocks[b])
            mlos.append(mlo)

        cell_list = np.concatenate(cells_t)          # 64 cells
        ch = cell_list
        zk0 = (lin[0] + k0[ch, :].astype(f32) * dgrid).astype(f32)
        A = (ci * y1[ch, :] - si * zk0).astype(f32).reshape(-1)
        B = (si * y1[ch, :] + ci * zk0).astype(f32).reshape(-1)
        rx2 = (rotx[ch, :] ** 2 + _EPS_R2D2).astype(f32).reshape(-1)
        Cc = Cp[ch, :].astype(f32).reshape(-1)

        pk = np.empty((128, 4 * COLS), dtype=f32)
        for ti, arr in enumerate((A, B, rx2, Cc)):
            pk[:64, ti * COLS:(ti + 1) * COLS] = arr[0::2][None, :]
            pk[64:, ti * COLS:(ti + 1) * COLS] = arr[1::2][None, :]
        in_maps.append({"pk": pk, "sm": sm, "ones": ones})
        core_meta.append((cell_list, mlos))

    return (S0, S1), in_maps, core_meta, (sig, dz)


def _run(key, in_maps, trace=False, **kwargs):
    from concourse.bass_utils import run_bass_kernel_spmd
    if key not in _CACHE:
        _CACHE[key] = _build_program(*key)
    return run_bass_kernel_spmd(_CACHE[key], in_maps,
                                list(range(N_CORES)), trace=trace, **kwargs)


def _assemble(results, key, core_meta, scale_info):
    f32 = np.float32
    S0, S1 = key
    sig, dz = scale_info
    cmag = np.sqrt(np.pi) * sig / (2.0 * dz)
    pref = 1.0 / np.sqrt(2.0 * np.pi * sig * sig)
    scale = f32(cmag * pref / (VEL_UP * IMG_UP * IMG_UP))

    out_half = np.zeros((N_GROUPS, 16, 32), dtype=np.float64)
    for core, r in enumerate(results):
        raw = np.asarray(r["out"])                   # [nrows, COLS]
        cell_list, mlos = core_meta[core]
        for t, Sx in ((0, S0), (1, S1)):
            cols = raw[:, t * TCOLS:(t + 1) * TCOLS]
            pooled = cols.reshape(-1, CELLS_PER_TILE, 8).sum(axis=2)
            mlo = mlos[t]
            rb = 0 if t == 0 else S0 + 1
            cells = cell_list[t * CELLS_PER_TILE:(t + 1) * CELLS_PER_TILE]
            for g in range(Sx + 1):
                p = mlo - 1 + g
                if p < 0 or p >= N_GROUPS:
                    continue
                for ci_i, cell in enumerate(cells):
                    out_half[p, cell // 32, cell % 32] += pooled[rb + g, ci_i]
    out_half = (out_half * scale).astype(f32)
    full = np.empty((N_GROUPS, 32, 32), dtype=f32)
    full[:, :16, :] = out_half
    full[:, 16:, :] = out_half[::-1, ::-1, ::-1]
    return full


def kernel(inclination, sky_rot, line_broadening):
    key, in_maps, core_meta, scale_info = _plan(
        inclination, sky_rot, line_broadening)
    res = _run(key, in_maps)
    return _assemble(res.results, key, core_meta, scale_info)


# revision 31
# speedup vs baseline: 1.0369x; 1.0369x over previous
"""Trainium2 Bass kernel for nn_CubeSimulator.

Reference computation: a 128^3 spatial grid is rotated (Rz(sky_rot) then
Rx(inclination)), a rotation-curve velocity field and an exponential-disk
intensity field are evaluated, an 80-channel Gaussian KDE over the
line-of-sight velocity reduces the third grid axis, and the [80,128,128]
cube is avg-pooled (5,4,4) to [16,32,32].

Kernel strategy (v3)
--------------------
* erf-collapsed KDE: a 5-channel pooled group sum of Gaussians at spacing
  dz << sigma is a midpoint-rule sum, equal to an erf difference with
  aliasing error ~e^{-pi^2 sig^2/dz^2} once the erf width is corrected to
  sig_e = sqrt(sig^2 - dz^2/6):
      sum_{r=0..4} exp(-(z_{5m+r}-vz)^2/sig^2)
        ~= C * [erf((e_{m+1}-vz)/sig_e) - erf((e_m-vz)/sig_e)]
  (~3e-5 max error).  80 exp channels collapse to <=15 erf evaluations.
* edge-slot sparsity: bf16 erf saturates to +-1 for |arg|>2.2, so per
  column tile only edges within the tile's vz range (+- margin) matter;
  live edges form a contiguous range [mlo..mhi].  The SPMD program has S =
  max-over-cores live-edge *slots* per tile; each core loads its own edge
  values into the sm bias columns and the host un-maps psum slots to
  physical velocity groups per core.  Cells are re-sharded globally by vz
  so each (core, tile) has a narrow vz span.
* +- PE accumulation: per edge slot only P_s = erf_s * src is formed; the
  group slot between slots s-1,s accumulates (+P_s) and (-P_{s-1}) via
  +1/-1 stationary vectors; boundary slots use (+src) for the saturated
  side.  PSUM start/stop handles the velocity pooling, no elementwise
  differences.
* k-window packing: src has a Gaussian vertical profile; per sky column a
  64-step |rot_z| window holds all non-negligible intensity.  Two sky
  points of one output pool cell pack into one 128-partition column.
* Point symmetry: (i,j,k) -> (-i,-j,-k) negates vz and preserves src, so
  only rows i<64 are computed; the host mirrors the pooled output.
"""

import sys

for _p in ("/opt/trn_rl_repo",):
    if _p not in sys.path:
        sys.path.insert(0, _p)

import numpy as np
import ml_dtypes

# ---------------- problem constants (compile-time, model-intrinsic) --------
IMAGE_RES = 128
VEL_RES = 80
VEL_UP = 5
IMG_UP = 4
N_CORES = 8
HALF_I = IMAGE_RES // 2            # 64 computed rows
KWIN = 64                          # k-window length (2 points/column)
COLS = 512                         # packed columns per core (1024 points)
TCOLS = COLS // 2                  # columns per sparsity tile
CELLS_PER_TILE = 32
CUBE_FOV = 1000.0
M_TO_PC = 1.0 / 3.086e16
V_MAX_PC = np.float32(200000.0 * M_TO_PC)
R_C = np.float32(0.1 * CUBE_FOV)
R_D = np.float32(0.3 * CUBE_FOV)
H_Z = np.float32(0.05 * CUBE_FOV)
VEL_MIN = -300000.0
VEL_MAX = 300000.0
N_GROUPS = VEL_RES // VEL_UP       # 16
N_EDGES = N_GROUPS + 1             # 17
MARGIN = 2.5                       # erf-saturation margin (sig_e units)

_INV_RD2 = 1.0 / (float(R_D) * float(R_D))
_NEG_H = float(-1.0 / (2.0 * float(H_Z) * float(H_Z)))
_RC2 = float(R_C) * float(R_C)
_EPS_R2D2 = np.float32(1e-25)

# sm scalar-column layout: [ nsz | ciz | rc2 | negh | ird2 | edge slots... ]
_C_NSZ = 0
_C_CIZ = 1
_C_RC2 = 2
_C_NEGH = 3
_C_IRD2 = 4
_C_EDG = 5                 # then S0 cols for tile0 slots, S1 for tile1

_CACHE = {}


def _build_program(S0, S1):
    from concourse import bacc, mybir, tile

    f32 = mybir.dt.float32
    bf16 = mybir.dt.bfloat16
    AF = mybir.ActivationFunctionType
    OP = mybir.AluOpType

    SM_COLS = _C_EDG + S0 + S1
    SLOTS = (S0, S1)
    # globally unique psum regions: tile0 slots 0..S0, tile1 S0+1..S0+S1+1
    n_regions = (S0 + 1) + (S1 + 1)
    assert n_regions <= 24, "edge-slot count exceeds PSUM capacity"
    nbanks = (n_regions + 2) // 3
    nrows_out = 3 * nbanks
    rbase = (0, S0 + 1)

    nc = bacc.Bacc(None)

    pk_d = nc.dram_tensor("pk", [128, 4 * COLS], f32, kind="ExternalInput")
    sm_d = nc.dram_tensor("sm", [128, SM_COLS], f32, kind="ExternalInput")
    ones_d = nc.dram_tensor("ones", [128, 64], bf16, kind="ExternalInput")
    out_d = nc.dram_tensor("out", [nrows_out, COLS], f32,
                           kind="ExternalOutput")

    with tile.TileContext(nc) as tc:
        with (
            tc.tile_pool(name="inp", bufs=1) as inp,
            tc.tile_pool(name="fld", bufs=1) as fld,
            tc.tile_pool(name="ep", bufs=4) as ep,
            tc.tile_pool(name="pp", bufs=6) as pp,
            tc.tile_pool(name="psum", bufs=1, space="PSUM") as psum,
            tc.tile_pool(name="ob", bufs=4) as obp,
        ):
            pk = inp.tile([128, 4 * COLS], f32)
            sm = inp.tile([128, SM_COLS], f32)
            ones = inp.tile([128, 64], bf16)
            # pk quarters spread over three queues, ordered by need
            for _c, _q in ((0, nc.sync), (2, nc.gpsimd), (1, nc.sync),
                           (3, nc.scalar)):
                _q.dma_start(pk[:, _c * COLS:(_c + 1) * COLS],
                             pk_d[:, _c * COLS:(_c + 1) * COLS])
            nc.gpsimd.dma_start(sm[:], sm_d[:])
            nc.gpsimd.dma_start(ones[:], ones_d[:])

            pa = pk[:, 0 * COLS:1 * COLS]
            pb = pk[:, 1 * COLS:2 * COLS]
            prx2 = pk[:, 2 * COLS:3 * COLS]
            pc = pk[:, 3 * COLS:4 * COLS]

            def col(i):
                return sm[:, i:i + 1]

            V = nc.vector
            G = nc.gpsimd
            S = nc.scalar

            roty = fld.tile([128, COLS], f32)
            rotz = fld.tile([128, COLS], f32)
            y2 = fld.tile([128, COLS], f32)
            r2d2 = fld.tile([128, COLS], f32)
            z2 = fld.tile([128, COLS], f32)
            q = fld.tile([128, COLS], f32)
            qc = fld.tile([128, COLS], f32)
            den = fld.tile([128, COLS], f32)
            rec = fld.tile([128, COLS], f32)
            u2 = fld.tile([128, 2 * COLS], f32)    # [ u | r2d2*invRD2 ]
            sus2 = fld.tile([128, 2 * COLS], f32)  # [ su | slq ]
            sarg = fld.tile([128, COLS], f32)
            sg = fld.tile([128, COLS], f32)
            tr = fld.tile([128, COLS], f32)
            vzt = fld.tile([128, COLS], f32)
            src = fld.tile([128, COLS], bf16)
            dummy = fld.tile([128, COLS], bf16)

            G.memset(dummy[:], 0.0)
            # pre-trigger the Sqrt act-table load while DMAs are in flight
            S.activation(sarg[:, 0:1], sm[:, _C_RC2:_C_RC2 + 1], AF.Sqrt)

            HALF = COLS // 2
            CHUNKS = ((0, G), (1, V))

            def sl_of(c):
                return slice(c * HALF, (c + 1) * HALF)

            for c, E in CHUNKS:
                s = sl_of(c)
                E.tensor_scalar_add(roty[:, s], pa[:, s], col(_C_NSZ))
                E.tensor_scalar_add(rotz[:, s], pb[:, s], col(_C_CIZ))
                E.tensor_mul(y2[:, s], roty[:, s], roty[:, s])
                E.tensor_add(r2d2[:, s], y2[:, s], prx2[:, s])
                E.tensor_mul(z2[:, s], rotz[:, s], rotz[:, s])
                E.tensor_add(q[:, s], r2d2[:, s], z2[:, s])
                if E is V:
                    E.scalar_tensor_tensor(den[:, s], q[:, s], col(_C_RC2),
                                           r2d2[:, s], op0=OP.add,
                                           op1=OP.mult)
                else:
                    E.tensor_scalar_add(qc[:, s], q[:, s], col(_C_RC2))
                    E.tensor_mul(den[:, s], qc[:, s], r2d2[:, s])

            # PE warmup + zero-init every used psum region (skipped group
            # slots keep these zeros; real slots reset via start=True)
            psb = [psum.tile([128, COLS], f32, name=f"acc{b}")
                   for b in range(nbanks)]
            for b in range(nbanks):
                for r in range(3):
                    nc.tensor.matmul(psb[b][32 * r:32 * r + 32, :],
                                     ones[:, 0:32], dummy[:, :],
                                     start=True, stop=True,
                                     skip_group_check=True)

            # u2 layout per chunk c: [ u_c (256) | r2d2_c/R_D^2 (256) ] so
            # each half can take one contiguous sqrt as soon as it's ready.
            def u_sl(c):
                return slice(c * COLS, c * COLS + HALF)

            def r_sl(c):
                return slice(c * COLS + HALF, (c + 1) * COLS)

            for c, _E in CHUNKS:
                # idle-window ACT; Copy is in every act table
                S.activation(u2[:, r_sl(c)], r2d2[:, sl_of(c)], AF.Copy,
                             scale=_INV_RD2)
            for c, _E in CHUNKS:
                s = sl_of(c)
                V.reciprocal_approx_fast(rec[:, s], den[:, s])
            for c, E in CHUNKS:
                s = sl_of(c)
                E.tensor_mul(u2[:, u_sl(c)], q[:, s], rec[:, s])
                # [su_c | slq_c] = sqrt([u_c | r2d2_c/R_D^2]), Sqrt preloaded
                S.activation(sus2[:, c * COLS:(c + 1) * COLS],
                             u2[:, c * COLS:(c + 1) * COLS], AF.Sqrt)

            def su_part(c):
                return sus2[:, c * COLS:c * COLS + HALF]

            def slq_part(c):
                return sus2[:, c * COLS + HALF:(c + 1) * COLS]

            for c, E in CHUNKS:
                s = sl_of(c)
                # sarg first so the src sigmoid becomes ready early
                if E is V:
                    E.scalar_tensor_tensor(sarg[:, s], z2[:, s], col(_C_NEGH),
                                           slq_part(c), op0=OP.mult,
                                           op1=OP.subtract)
                else:
                    E.tensor_scalar_mul(qc[:, s], z2[:, s], col(_C_NEGH))
                    E.tensor_sub(sarg[:, s], qc[:, s], slq_part(c))

            # src = exp(sarg) via sigmoid (erf's own act table -> only one
            # mid-stream table switch): e^-s = 1/sigmoid(s) - 1
            # per tile so tile0's erf products unblock immediately
            for t in (0, 1):
                ts_ = slice(t * TCOLS, (t + 1) * TCOLS)
                S.activation(sg[:, ts_], sarg[:, ts_], AF.Sigmoid, scale=-1.0)
                V.reciprocal_approx_fast(tr[:, ts_], sg[:, ts_])
                G.tensor_scalar_add(src[:, ts_], tr[:, ts_], -1.0)

            for c, E in CHUNKS:
                s = sl_of(c)
                E.tensor_mul(vzt[:, s], su_part(c), pc[:, s])

            # ---- KDE: erf edge slots + (+-1)-stationary PSUM streams ----
            onesp = ones[:, 0:32]
            onesn = ones[:, 32:64]

            def tslice(t):
                return slice(t * TCOLS, (t + 1) * TCOLS)

            # group slot (t, g) -> psum bank g//3, rows 32*(g%3), tile cols
            started = set()

            def mm(t, g, stat, mov, stop):
                R = rbase[t] + g
                b, r = R // 3, R % 3
                st = (t, g) not in started
                started.add((t, g))
                nc.tensor.matmul(
                    psb[b][32 * r:32 * r + 32, tslice(t)], stat, mov,
                    start=st, stop=stop, skip_group_check=True)

            # bank completion bookkeeping
            bank_need = [0] * nbanks
            for t in (0, 1):
                for g in range(SLOTS[t] + 1):
                    bank_need[(rbase[t] + g) // 3] += 1
            bank_got = [0] * nbanks
            banks_done = [0]
            deferred = []

            def emit_bank_out(b, on_act, q):
                # each bank's regions belong to a single tile -> only move
                # that tile's column half (the other half is warmup zeros)
                tset = {0 if r <= S0 else 1 for r in range(3 * b, 3 * b + 3)
                        if r < n_regions}
                cs = tslice(tset.pop()) if len(tset) == 1 else slice(0, COLS)
                ot = obp.tile([128, COLS], f32, tag="ob", name=f"ot{b}")
                if on_act:
                    # ACT is idle after the last erf
                    S.activation(ot[0:96, cs], psb[b][0:96, cs], AF.Copy)
                else:
                    V.tensor_copy(ot[0:96, cs], psb[b][0:96, cs])
                q.dma_start(out_d[3 * b:3 * b + 3, cs], ot[0:96:32, cs])

            def note_stop(t, g):
                b = (rbase[t] + g) // 3
                bank_got[b] += 1
                if bank_got[b] == bank_need[b]:
                    banks_done[0] += 1
                    # defer the last two banks so their copies don't steal
                    # DVE priority from the final erf products
                    if banks_done[0] <= nbanks - 2:
                        emit_bank_out(b, False,
                                      (nc.sync, nc.gpsimd)[banks_done[0] % 2])
                    else:
                        deferred.append(b)

            # group slot 0 and the final slot S_t of each tile get their
            # (+src) streams as soon as src exists; the final slot is then
            # closed by its (-P) stream right after the last erf product
            for t in (0, 1):
                mm(t, 0, onesp, src[:, tslice(t)], False)
                mm(t, SLOTS[t], onesp, src[:, tslice(t)], False)

            nerf = 0
            for t in (0, 1):
                ts = tslice(t)
                for s_idx in range(SLOTS[t]):
                    E_t = ep.tile([128, TCOLS], bf16, tag=f"E{t}")
                    S.activation(E_t[:], vzt[:, ts], AF.Erf,
                                 bias=col(_C_EDG + (0 if t == 0 else S0)
                                          + s_idx),
                                 scale=-1.0)
                    P_t = pp.tile([128, TCOLS], bf16, tag=f"P{t}")
                    n_left = (S0 + S1) - nerf
                    eng = V if n_left == 1 else (V if nerf % 2 else G)
                    eng.tensor_mul(P_t[:], E_t[:], src[:, ts])
                    nerf += 1
                    # +P_s closes group slot s; -P_s joins group slot s+1
                    # (already opened by +src for the final slot)
                    mm(t, s_idx, onesp, P_t[:, :], True)
                    note_stop(t, s_idx)
                    mm(t, s_idx + 1, onesn, P_t[:, :],
                       s_idx + 1 == SLOTS[t])
                    if s_idx + 1 == SLOTS[t]:
                        note_stop(t, SLOTS[t])
            for i, b in enumerate(deferred):
                # DVE copies (it is free after the last erf product);
                # alternate DMA queues, last on the faster sync queue
                emit_bank_out(b, False, (nc.gpsimd, nc.sync)[i % 2])

    nc.finalize()
    return nc


def _plan(inclination, sky_rot, line_broadening):
    """Host-side per-input planning: cell vz ranges, global re-sharding,
    per-(core,tile) edge windows, packed input tensors."""
    f32 = np.float32
    inc = f32(inclination)
    rot = f32(sky_rot)
    lb = f32(line_broadening)
    ci, si = f32(np.cos(inc)), f32(np.sin(inc))
    cr, sr = f32(np.cos(rot)), f32(np.sin(rot))

    lin = np.linspace(-CUBE_FOV, CUBE_FOV, IMAGE_RES, dtype=f32)
    dgrid = f32(lin[1] - lin[0])
    zl = np.linspace(f32(VEL_MIN * M_TO_PC), f32(VEL_MAX * M_TO_PC),
                     VEL_RES, dtype=f32)
    dz = float(zl[-1] - zl[0]) / (VEL_RES - 1)
    sig = float(lb)
    sig_e = f32(np.sqrt(sig * sig - dz * dz / 6.0))

    # dropped vertical src mass outside the 64-step window; measured output
    # error stays ~20x below it (drops hit dim pixels), so 2e-2 here keeps
    # rel err under ~1.5e-2 even for near-edge-on inclinations.
    t_keep = (KWIN / 2) * abs(float(ci)) * float(dgrid)
    eps_drop = np.exp(-t_keep * t_keep / (2.0 * float(H_Z) ** 2))
    if eps_drop > 2e-2:
        raise RuntimeError(
            f"k-window packing invalid for inclination={inc} "
            f"(eps_drop={eps_drop:.2e}); phi=1 fallback not built")

    edges = np.empty(N_EDGES, dtype=np.float64)
    edges[0] = zl[0] - dz / 2
    for m in range(1, N_GROUPS):
        edges[m] = (float(zl[5 * m - 1]) + float(zl[5 * m])) / 2
    edges[N_GROUPS] = zl[-1] + dz / 2
    edges = (edges - edges[::-1]) / 2
    ep_n = edges / float(sig_e)          # edge values in vzt units

    vmax_proj = abs(float(si)) * float(V_MAX_PC)
    margin = (float(edges[N_GROUPS]) - vmax_proj) / float(sig_e)
    if margin < 3.0:     # bf16 erf saturates to exactly 1.0 beyond ~2.2
        raise RuntimeError(
            f"outer erf edge not saturated (margin={margin:.2f} sigma)")

    # ---- per-cell geometry and vz ranges ----
    io = np.repeat(np.arange(16), 32)
    jo = np.tile(np.arange(32), 16)
    di = np.repeat(np.arange(4), 4)[None, :]
    dj = np.tile(np.arange(4), 4)[None, :]
    xi = lin[(io[:, None] * 4 + di)].astype(f32)     # [512, 16]
    yj = lin[(jo[:, None] * 4 + dj)].astype(f32)
    y1 = (sr * xi + cr * yj).astype(f32)
    rotx = (cr * xi - sr * yj).astype(f32)
    kc = (-si * y1 / ci - lin[0]) / dgrid
    k0 = np.clip(np.round(kc - KWIN / 2), 0,
                 IMAGE_RES - KWIN).astype(np.int64)
    pidx = np.arange(KWIN, dtype=f32)
    zk = (lin[0] + (k0[..., None] + pidx) * dgrid).astype(f32)  # [512,16,64]
    roty_h = (ci * y1)[..., None] - si * zk
    rotz_h = (si * y1)[..., None] + ci * zk
    r2d2_h = roty_h * roty_h + (rotx * rotx)[..., None] + 1e-25
    q_h = r2d2_h + rotz_h * rotz_h
    su_h = np.sqrt(q_h / ((q_h + float(R_C) ** 2) * r2d2_h))
    Cp = (-si * V_MAX_PC * rotx / sig_e).astype(f32)
    vzt_h = Cp[..., None] * su_h                     # [512, 16, 64]
    cell_min = vzt_h.min(axis=(1, 2))
    cell_max = vzt_h.max(axis=(1, 2))

    # ---- global sort, blocks of 32 cells, S per block ----
    order = np.argsort(cell_min + cell_max, kind="stable")
    blocks = [order[32 * b:32 * b + 32] for b in range(16)]

    def block_window(cells_idx):
        vmin = cell_min[cells_idx].min()
        vmax = cell_max[cells_idx].max()
        live = [m for m in range(1, N_GROUPS)
                if vmin - MARGIN <= ep_n[m] <= vmax + MARGIN]
        if live:
            return live[0], live[-1]
        # vz range inside one group: first edge above the range
        m = int(np.searchsorted(ep_n, vmax + MARGIN))
        m = min(max(m, 1), N_GROUPS - 1)
        return m, m

    wins = [block_window(b) for b in blocks]
    sizes = [w[1] - w[0] + 1 for w in wins]
    # big-S blocks to tile slot 0 (one per core), small to slot 1
    bo = sorted(range(16), key=lambda b: -sizes[b])
    slot_blocks = [bo[:8], bo[8:]]
    S0 = max(sizes[b] for b in slot_blocks[0])
    S1 = max(sizes[b] for b in slot_blocks[1])

    SM_COLS = _C_EDG + S0 + S1
    sm_base = np.zeros((128, SM_COLS), dtype=f32)
    pmod = (np.arange(128) % KWIN).astype(f32)
    sm_base[:, _C_NSZ] = (-si * dgrid) * pmod
    sm_base[:, _C_CIZ] = (ci * dgrid) * pmod
    sm_base[:, _C_RC2] = f32(_RC2)
    sm_base[:, _C_NEGH] = f32(_NEG_H)
    sm_base[:, _C_IRD2] = f32(_INV_RD2)

    ones = np.empty((128, 64), dtype=ml_dtypes.bfloat16)
    ones[:, 0:32] = 1.0
    ones[:, 32:64] = -1.0

    in_maps = []
    core_meta = []
    for core in range(N_CORES):
        sm = sm_base.copy()
        cells_t = []
        mlos = []
        for t, Sx in ((0, S0), (1, S1)):
            b = slot_blocks[t][core]
            mlo, mhi = wins[b]
            while mhi - mlo + 1 < Sx:
                if mhi < N_GROUPS - 1:
                    mhi += 1
                else:
                    mlo -= 1
            assert mlo >= 1 and mhi <= N_GROUPS - 1
            base = _C_EDG if t == 0 else _C_EDG + S0
            for s_i in range(Sx):
                sm[:, base + s_i] = f32(ep_n[mlo + s_i])
            cells_t.append(blocks[b])
            mlos.append(mlo)

        cell_list = np.concatenate(cells_t)          # 64 cells
        ch = cell_list
        zk0 = (lin[0] + k0[ch, :].astype(f32) * dgrid).astype(f32)
        A = (ci * y1[ch, :] - si * zk0).astype(f32).reshape(-1)
        B = (si * y1[ch, :] + ci * zk0).astype(f32).reshape(-1)
        rx2 = (rotx[ch, :] ** 2 + _EPS_R2D2).astype(f32).reshape(-1)
        Cc = Cp[ch, :].astype(f32).reshape(-1)

        pk = np.empty((128, 4 * COLS), dtype=f32)
        for ti, arr in enumerate((A, B, rx2, Cc)):
            pk[:64, ti * COLS:(ti + 1) * COLS] = arr[0::2][None, :]
            pk[64:, ti * COLS:(ti + 1) * COLS] = arr[1::2][None, :]
        in_maps.append({"pk": pk, "sm": sm, "ones": ones})
        core_meta.append((cell_list, mlos))

    return (S0, S1), in_maps, core_meta, (sig, dz)


def _run(key, in_maps, trace=False, **kwargs):
    from concourse.bass_utils import run_bass_kernel_spmd
    if key not in _CACHE:
        _CACHE[key] = _build_program(*key)
    return run_bass_kernel_spmd(_CACHE[key], in_maps,
                                list(range(N_CORES)), trace=trace, **kwargs)


def _assemble(results, key, core_meta, scale_info):
    f32 = np.float32
    S0, S1 = key
    sig, dz = scale_info
    cmag = np.sqrt(np.pi) * sig / (2.0 * dz)
    pref = 1.0 / np.sqrt(2.0 * np.pi * sig * sig)
    scale = f32(cmag * pref / (VEL_UP * IMG_UP * IMG_UP))

    out_half = np.zeros((N_GROUPS, 16, 32), dtype=np.float64)
    for core, r in enumerate(results):
        raw = np.asarray(r["out"])                   # [nrows, COLS]
        cell_list, mlos = core_meta[core]
        for t, Sx in ((0, S0), (1, S1)):
            cols = raw[:, t * TCOLS:(t + 1) * TCOLS]
            pooled = cols.reshape(-1, CELLS_PER_TILE, 8).sum(axis=2)
            mlo = mlos[t]
            rb = 0 if t == 0 else S0 + 1
            cells = cell_list[t * CELLS_PER_TILE:(t + 1) * CELLS_PER_TILE]
            for g in range(Sx + 1):
                p = mlo - 1 + g
                if p < 0 or p >= N_GROUPS:
                    continue
                for ci_i, cell in enumerate(cells):
                    out_half[p, cell // 32, cell % 32] += pooled[rb + g, ci_i]
    out_half = (out_half * scale).astype(f32)
    full = np.empty((N_GROUPS, 32, 32), dtype=f32)
    full[:, :16, :] = out_half
    full[:, 16:, :] = out_half[::-1, ::-1, ::-1]
    return full


def kernel(inclination, sky_rot, line_broadening):
    key, in_maps, core_meta, scale_info = _plan(
        inclination, sky_rot, line_broadening)
    res = _run(key, in_maps)
    return _assemble(res.results, key, core_meta, scale_info)
